# revision 10
# baseline (speedup 1.0000x reference)
"""CapsuleModel2 segment-reduce kernel for 8 TRN2 NeuronCores (v3: ap_gather).

Math (per reference.py):
    feats  = class_capsules.reshape(65536, 272)[point_idx]        # [P, 272]
    sums   = segment_sum(feats, segment_ids, 4096)                # [4096, 272]
    counts = segment_sum(ones)                                    # [4096]
    out    = sigmoid((sums / max(counts,1)) @ W + b)              # [4096, 19]

Key identity: (sums @ W) = segment_sum(feats @ W) — project the grid down to
19 channels FIRST, then reduce per segment.

The projected table lives IN SBUF, channels-on-partitions ([19, 8192] f32 on
partitions 0..18, + a zero col 8192 for padding). Point rows are fetched with
gpsimd.ap_gather (SBUF-local free-dim gather, measured ~0.5us per call — no
DMA descriptors at all, which is what killed v1's dma_gather at 530us).

Per core (owns grid cells [k*8192,(k+1)*8192) and the points hitting them):
  A) transposed projection: psum[19, 512] = W16^T @ gridT per 512-cell tile,
     copied straight into the SBUF table (no DRAM table, no transposes)
  B) 16 gather calls x 4608 points: ap_gather -> cast bf16 -> per 128-point
     chunk PE-transpose [19,128]->[128,19] into a batched PSUM tile
     ([128,18,19], ONE scalar copy per 18 chunks) -> one-hot segment matmul
     psum[64, 4win, 20] += oh^T @ [X | 1]  (col 19 = segment count;
     padding gathers the zero col and has segrel=-1 -> contributes nothing)
  C) partial[64seg%64, 64win, 20] f16 stored p-major (contiguous per
     partition), ONE ReduceScatter(add) splits by partition rows: core k owns
     segments with s%64 in [8k, 8k+8); finalize divides by count, undoes the
     x16 fp8 W scale, bias, sigmoid. Host reorders rows on assemble.
"""

import sys

for _p in ('/opt/trn_rl_repo',):
    if _p not in sys.path:
        sys.path.insert(0, _p)

import numpy as np
import ml_dtypes

import concourse.bacc as bacc
import concourse.mybir as mybir
import concourse.tile as tile

BF16 = mybir.dt.bfloat16
F32 = mybir.dt.float32
I16 = mybir.dt.int16
F16 = mybir.dt.float16
F8 = mybir.dt.float8e4

NCORE = 8
GRID = 65536
GPC = GRID // NCORE          # 8192 grid rows (cells) per core
D = 272                      # capsule feature dim
NCH = 19                     # output channels
NW = NCH + 1                 # + count column in the reduce output
NSEG = 4096
WIN = 64                     # segments per window (one-hot width)
NWIN = NSEG // WIN           # 64 windows
CAP = 1152                   # padded points per (core, window); actual max ~1118
CPW = CAP // 128             # 9 chunks per window
NCHUNK = NWIN * CPW          # 576 chunks per core
NIDX = NCHUNK * 128          # 73728 gather slots per core
NCALL = 16                   # gather calls (pipeline granularity)
SPC = NIDX // NCALL          # 4608 slots per call
CPC = SPC // 128             # 36 chunks per call
WPC = CPC // CPW             # 4 windows per call
MT = 512                     # cells per projection matmul
NMT = GPC // MT              # 16
ZCOL = GPC                   # zero column index (padding gather target)
WSCALE = 16.0                # fp8 W pre-scale; undone in finalize
QT = 18                      # transposes batched per PSUM tile / scalar copy


def build_nc(skip_collective=False):
    nc = bacc.Bacc("TRN2", num_devices=NCORE)

    gridT = nc.dram_tensor("gridT", [D, GPC], F8, kind="ExternalInput")
    w_pack = nc.dram_tensor("w_pack", [128, 3 * NCH], F8, kind="ExternalInput")
    idx_in = nc.dram_tensor("idx", [32, NCALL, SPC // 16], I16,
                            kind="ExternalInput")
    segrel_in = nc.dram_tensor("segrel", [128, NCHUNK], BF16,
                               kind="ExternalInput")
    iota_in = nc.dram_tensor("iota", [128, WIN], BF16, kind="ExternalInput")
    bias_in = nc.dram_tensor("bias", [128, NCH], F32, kind="ExternalInput")
    eye_in = nc.dram_tensor("eye", [128, 128], BF16, kind="ExternalInput")
    if skip_collective:
        out_t = nc.dram_tensor("out", [64, NWIN, NW], F16,
                               kind="ExternalOutput")
    else:
        out_t = nc.dram_tensor("out", [8, NWIN, NCH], F32,
                               kind="ExternalOutput")

    partial_d = nc.dram_tensor("partial", [64, NWIN, NW], F16)
    rs_out = nc.dram_tensor("rs_out", [8, NWIN, NW], F16)

    ksizes = [(0, 128), (128, 128), (256, 16)]

    with tile.TileContext(nc) as tc:
        with (
            tc.tile_pool(name="const", bufs=1) as cpool,
            tc.tile_pool(name="tabp", bufs=1) as tabpool,
            tc.tile_pool(name="grid", bufs=1) as gpool,
            tc.tile_pool(name="proj", bufs=2, space="PSUM") as prpool,
            tc.tile_pool(name="dst", bufs=2) as dpool,
            tc.tile_pool(name="dstb", bufs=2) as bpool,
            tc.tile_pool(name="gsb", bufs=2) as spool,
            tc.tile_pool(name="oh", bufs=2) as opool,
            tc.tile_pool(name="pt", bufs=3, space="PSUM") as ptpool,
            tc.tile_pool(name="pblk", bufs=2, space="PSUM") as pbpool,
            tc.tile_pool(name="part", bufs=2) as papool,
            tc.tile_pool(name="fin", bufs=2) as fpool,
        ):
            # constants
            w_sb = cpool.tile([128, 3, NCH], F8)
            nc.sync.dma_start(w_sb[:], w_pack[:].rearrange(
                "p (t c) -> p t c", t=3))
            segrel_sb = cpool.tile([128, NCHUNK], BF16)
            nc.sync.dma_start(segrel_sb[:], segrel_in[:])
            iota_sb = cpool.tile([128, WIN], BF16)
            nc.sync.dma_start(iota_sb[:], iota_in[:])
            bias_sb = cpool.tile([128, NCH], F32)
            nc.sync.dma_start(bias_sb[:], bias_in[:])
            eye_sb = cpool.tile([128, 128], BF16)
            nc.sync.dma_start(eye_sb[:], eye_in[:])
            idx_sb = cpool.tile([32, NCALL, SPC // 16], I16)
            nc.sync.dma_start(idx_sb[:], idx_in[:])

            # ---- Phase A: transposed projection -> SBUF table ----
            table = tabpool.tile([32, GPC + 1], F32)
            nc.vector.memset(table[:], 0.0)
            gt = gpool.tile([128, 3, GPC], F8)
            for t, (k0, kn) in enumerate(ksizes):
                nc.sync.dma_start(gt[:kn, t, :], gridT[k0:k0 + kn, :])
            for mt in range(NMT):
                psum = prpool.tile([NCH, MT], F32, tag="proj")
                for t, (k0, kn) in enumerate(ksizes):
                    nc.tensor.matmul(
                        out=psum[:],
                        lhsT=w_sb[:kn, t, :],
                        rhs=gt[:kn, t, mt * MT:(mt + 1) * MT],
                        start=(t == 0), stop=(t == 2))
                nc.scalar.copy(table[:NCH, mt * MT:(mt + 1) * MT], psum[:])

            # ---- Phase B: ap_gather + transpose + one-hot reduce ----
            for i in range(NCALL):
                dst = dpool.tile([32, SPC], F32, tag="dst")
                nc.gpsimd.ap_gather(
                    out_ap=dst[:], in_ap=table[:],
                    idxs_ap=idx_sb[:, i, :],
                    channels=32, num_elems=GPC + 1, d=1, num_idxs=SPC)
                dstb = bpool.tile([32, SPC], BF16, tag="dstb")
                nc.vector.tensor_copy(dstb[:], dst[:])

                oh = opool.tile([128, CPC, WIN], BF16, tag="oh")
                nc.vector.tensor_tensor(
                    out=oh[:],
                    in0=segrel_sb[:, i * CPC:(i + 1) * CPC]
                        .rearrange("p (c o) -> p c o", o=1).broadcast_to(
                            [128, CPC, WIN]),
                    in1=iota_sb[:].rearrange("p (o j) -> p o j", o=1)
                        .broadcast_to([128, CPC, WIN]),
                    op=mybir.AluOpType.is_equal)

                gsb = spool.tile([128, CPC, NW], BF16, tag="gsb")
                nc.vector.memset(gsb[:, :, NCH:NW], 1.0)
                for q in range(CPC // QT):
                    pt = ptpool.tile([128, QT, NW], BF16, tag="pt")
                    for t in range(QT):
                        c = q * QT + t
                        nc.tensor.matmul(
                            out=pt[:, t, :NCH],
                            lhsT=dstb[:NCH, c * 128:(c + 1) * 128],
                            rhs=eye_sb[:NCH, :NCH],
                            is_transpose=True)
                    nc.scalar.copy(gsb[:, q * QT:(q + 1) * QT, :NCH],
                                   pt[:, :, :NCH])

                psum_w = pbpool.tile([WIN, WPC, NW], F32, tag="pblk")
                for c in range(CPC):
                    h, j = divmod(c, CPW)
                    nc.tensor.matmul(
                        out=psum_w[:, h, :],
                        lhsT=oh[:, c, :],
                        rhs=gsb[:, c, :],
                        start=(j == 0), stop=(j == CPW - 1))
                part = papool.tile([WIN, WPC, NW], F16, tag="part")
                nc.scalar.copy(part[:], psum_w[:])
                nc.sync.dma_start(
                    (out_t if skip_collective else partial_d)
                        [:, WPC * i:WPC * (i + 1), :],
                    part[:])

            if not skip_collective:
                nc.gpsimd.collective_compute(
                    "ReduceScatter",
                    mybir.AluOpType.add,
                    replica_groups=[list(range(NCORE))],
                    ins=[partial_d[:]],
                    outs=[rs_out[:]],
                )

            # ---- Phase C: finalize ----
            if not skip_collective:
                fin16 = fpool.tile([8, NWIN, NW], F16, tag="fin16")
                nc.sync.dma_start(fin16[:], rs_out[:])
                fin = fpool.tile([8, NWIN, NW], F32, tag="fin")
                nc.vector.tensor_copy(fin[:], fin16[:])
                cnt = fpool.tile([8, NWIN, 1], F32, tag="cnt")
                nc.vector.tensor_scalar_max(cnt[:], fin[:, :, NCH:NW], 1.0)
                rec = fpool.tile([8, NWIN, 1], F32, tag="rec")
                nc.vector.reciprocal(rec[:], cnt[:])
                sc = fpool.tile([8, NWIN, NCH], F32, tag="sc")
                nc.vector.tensor_tensor(
                    out=sc[:], in0=fin[:, :, :NCH],
                    in1=rec[:].broadcast_to([8, NWIN, NCH]),
                    op=mybir.AluOpType.mult)
                # undo the x16 fp8 W scale, add bias, sigmoid
                sc2 = fpool.tile([8, NWIN, NCH], F32, tag="sc2")
                nc.vector.tensor_scalar_mul(sc2[:], sc[:], 1.0 / WSCALE)
                sc3 = fpool.tile([8, NWIN, NCH], F32, tag="sc3")
                nc.vector.tensor_tensor(
                    out=sc3[:], in0=sc2[:],
                    in1=bias_sb[:8].rearrange("p (h c) -> p h c", h=1)
                        .broadcast_to([8, NWIN, NCH]),
                    op=mybir.AluOpType.add)
                og = fpool.tile([8, NWIN, NCH], F32, tag="og")
                nc.scalar.activation(og[:], sc3[:],
                                     mybir.ActivationFunctionType.Sigmoid)
                nc.sync.dma_start(out_t[:], og[:])

    nc.compile()
    return nc


def prep_inputs(class_capsules, W, b, point_idx, segment_ids, num_segments=NSEG):
    """Host-side sharding: returns in_maps (list of 8 dicts)."""
    assert int(num_segments) == NSEG
    grid = np.ascontiguousarray(class_capsules.reshape(GRID, D), np.float32)
    point_idx = np.asarray(point_idx, np.int64)
    segment_ids = np.asarray(segment_ids, np.int64)
    W = np.asarray(W, np.float32)
    b = np.asarray(b, np.float32)

    f8 = ml_dtypes.float8_e4m3fn
    w_pack = np.zeros((128, 3 * NCH), f8)
    w16 = (W * WSCALE).astype(f8)
    w_pack[0:128, 0:NCH] = w16[0:128]
    w_pack[0:128, NCH:2 * NCH] = w16[128:256]
    w_pack[0:16, 2 * NCH:3 * NCH] = w16[256:272]

    iota = np.tile(np.arange(WIN, dtype=np.float32), (128, 1)).astype(
        ml_dtypes.bfloat16)
    bias_rep = np.tile(b[None, :], (128, 1)).astype(np.float32)
    eye = np.eye(128, dtype=np.float32).astype(ml_dtypes.bfloat16)

    in_maps = []
    for k in range(NCORE):
        sel = (point_idx >= k * GPC) & (point_idx < (k + 1) * GPC)
        lidx = (point_idx[sel] - k * GPC).astype(np.int64)
        lseg = segment_ids[sel]          # sorted ascending
        win = (lseg >> 6).astype(np.int64)
        srel = (lseg & 63).astype(np.float32)
        counts = np.bincount(win, minlength=NWIN)
        assert counts.max() <= CAP, f"core {k}: window count {counts.max()} > CAP"
        start = np.zeros(NWIN, np.int64)
        start[1:] = np.cumsum(counts)[:-1]
        rank = np.arange(lidx.size) - start[win]
        pos = win * CAP + rank

        idx_pad = np.full(NIDX, ZCOL, np.int16)   # padding -> zero column
        srel_pad = np.full(NIDX, -1.0, np.float32)
        idx_pad[pos] = lidx
        srel_pad[pos] = srel

        # idx wrapped per call: partition p (<16, dup at p+16), col s ->
        # idx_pad[i*SPC + s*16 + p]
        idxw = np.zeros((32, NCALL, SPC // 16), np.int16)
        for i in range(NCALL):
            blk = idx_pad[i * SPC:(i + 1) * SPC].reshape(-1, 16).T
            idxw[0:16, i] = blk
            idxw[16:32, i] = blk

        segrel_arr = np.ascontiguousarray(
            srel_pad.reshape(NCHUNK, 128).T).astype(ml_dtypes.bfloat16)

        gridT_k = np.ascontiguousarray(
            grid[k * GPC:(k + 1) * GPC].T).astype(f8)

        in_maps.append({
            "gridT": gridT_k,
            "w_pack": w_pack,
            "idx": idxw,
            "segrel": segrel_arr,
            "iota": iota,
            "bias": bias_rep,
            "eye": eye,
        })
    return in_maps


def assemble(results):
    # core k's rs slice holds segments s with s%64 in [8k, 8k+8):
    # out_k[r, w, :] = segment w*64 + 8k + r
    out = np.empty((NSEG, NCH), np.float32)
    for k in range(NCORE):
        res = results[k]["out"]           # [8, 64, 19]
        segs = (np.arange(NWIN)[None, :] * WIN + 8 * k
                + np.arange(8)[:, None])  # [8, 64]
        out[segs.ravel()] = res.reshape(-1, NCH)
    return out


_NC_CACHE = {}


def kernel(class_capsules, W, b, point_idx, segment_ids, num_segments):
    """Full-input entry point: shard across 8 NeuronCores, run, reassemble."""
    from concourse.bass_utils import run_bass_kernel_spmd

    in_maps = prep_inputs(np.asarray(class_capsules), np.asarray(W),
                          np.asarray(b), np.asarray(point_idx),
                          np.asarray(segment_ids), int(num_segments))
    if "nc" not in _NC_CACHE:
        _NC_CACHE["nc"] = build_nc()
    res = run_bass_kernel_spmd(_NC_CACHE["nc"], in_maps, list(range(NCORE)))
    return assemble(res.results)


# revision 11
# speedup vs baseline: 3.9597x; 3.9597x over previous
"""CapsuleModel2 segment-reduce kernel for 8 TRN2 NeuronCores (v4).

Math (per reference.py):
    feats  = class_capsules.reshape(65536, 272)[point_idx]        # [P, 272]
    sums   = segment_sum(feats, segment_ids, 4096)                # [4096, 272]
    counts = segment_sum(ones)                                    # [4096]
    out    = sigmoid((sums / max(counts,1)) @ W + b)              # [4096, 19]

Key identity: (sums @ W) = segment_sum(feats @ W) — project the 65536x272
grid down to 19 channels + a constant count column FIRST (fp8 matmul on the
PE), write the 256B-row table to DRAM, then dma_gather one row per point.

Distribution (table-sharded): core k owns grid cells [k*8192,(k+1)*8192) and
the points hitting them; partial sums over ALL 4096 segments; one
ReduceScatter(add). Points are binned per 64-segment window on host (window
padded to whole 128-point chunks) so each chunk's one-hot matmul
psum[64, 20] += oh^T @ rows covers one window.

v4 vs the 530us v1 baseline (same gather mechanism — SWDGE dma_gather at
~17ns/packet/queue is the floor):
  - 16 slices round-robin on all 4 SWDGE queues for steady transfer flow
  - one-hot matrices precomputed on HOST and DMA'd in (bf16) — no DVE
    is_equal builds on the critical path
  - partial sums stored p-major [64seg%64, 64win, 20] f16 (contiguous 160B
    per partition per store, vs 40B-descriptor sprays), per-slice `part`
    tiles from a bufs=2 pool so slice i+1 never waits on slice i's store
  - ReduceScatter splits by partition rows: core k owns segments with
    s%64 in [8k,8k+8); host reorders on assemble
  - batched psum->part copy (one scalar op per slice, not per window)
"""

import sys

for _p in ('/opt/trn_rl_repo',):
    if _p not in sys.path:
        sys.path.insert(0, _p)

import numpy as np
import ml_dtypes

import concourse.bacc as bacc
import concourse.mybir as mybir
import concourse.tile as tile

BF16 = mybir.dt.bfloat16
F32 = mybir.dt.float32
I16 = mybir.dt.int16
F16 = mybir.dt.float16
F8 = mybir.dt.float8e4

NCORE = 8
GRID = 65536
GPC = GRID // NCORE          # 8192 grid cells per core
D = 272                      # capsule feature dim
NCH = 19                     # output channels
NW = NCH + 1                 # + count column
NSEG = 4096
WIN = 64                     # segments per window (one-hot width)
NWIN = NSEG // WIN           # 64 windows
CAP = 1152                   # padded points per (core, window); actual max ~1118
CPW = CAP // 128             # 9 chunks per window
NCHUNK = NWIN * CPW          # 576 chunks per core
NIDX = NCHUNK * 128          # 73728 gather slots per core
NSLICE = 16
CPS = NCHUNK // NSLICE       # 36 chunks per slice
WPS = NWIN // NSLICE         # 4 windows per slice
IDX_PER_SLICE = NIDX // NSLICE   # 4608
ELEM = 128                   # table row width (bf16) = 256B (dma_gather min)
MTILE = 2048                 # grid cells per projection step
WSCALE = 16.0                # fp8 W pre-scale; cancels against count col


def build_nc(skip_collective=False):
    nc = bacc.Bacc("TRN2", num_devices=NCORE, num_swdge_queues=4)

    gridT = nc.dram_tensor("gridT", [D, GPC], F8, kind="ExternalInput")
    w_pack = nc.dram_tensor("w_pack", [128, 60], F8, kind="ExternalInput")
    idx_in = nc.dram_tensor("idx", [128, NSLICE, IDX_PER_SLICE // 16], I16,
                            kind="ExternalInput")
    oh_in = nc.dram_tensor("oh", [128, NCHUNK, WIN], BF16,
                           kind="ExternalInput")
    bias_in = nc.dram_tensor("bias", [128, NCH], F32, kind="ExternalInput")
    if skip_collective:
        out_t = nc.dram_tensor("out", [64, NWIN, NW], F16,
                               kind="ExternalOutput")
    else:
        out_t = nc.dram_tensor("out", [8, NWIN, NCH], F32,
                               kind="ExternalOutput")

    table = nc.dram_tensor("table", [GPC, ELEM], BF16)
    partial_d = nc.dram_tensor("partial", [64, NWIN, NW], F16)
    rs_out = nc.dram_tensor("rs_out", [8, NWIN, NW], F16)

    with tile.TileContext(nc) as tc:
        with (
            tc.tile_pool(name="const", bufs=1) as cpool,
            tc.tile_pool(name="grid", bufs=2) as gpool,
            tc.tile_pool(name="tab", bufs=2) as tpool,
            tc.tile_pool(name="ptab", bufs=2, space="PSUM") as pt_pool,
            tc.tile_pool(name="dst", bufs=3) as dpool,
            tc.tile_pool(name="pblk", bufs=2, space="PSUM") as pb_pool,
            tc.tile_pool(name="part", bufs=2) as papool,
            tc.tile_pool(name="fin", bufs=2) as fpool,
        ):
            # constants
            w_sb = cpool.tile([128, 60], F8)
            nc.sync.dma_start(w_sb[:], w_pack[:])
            bias_sb = cpool.tile([128, NCH], F32)
            nc.sync.dma_start(bias_sb[:], bias_in[:])
            idx_all = cpool.tile([128, NSLICE, IDX_PER_SLICE // 16], I16)
            nc.sync.dma_start(idx_all[:], idx_in[:])
            oh_sb = cpool.tile([128, NCHUNK, WIN], BF16)
            nc.sync.dma_start(oh_sb[:], oh_in[:])

            # ---- Phase A: projection -> DRAM table (256B bf16 rows) ----
            MS = MTILE // 128    # 16 psum chunks per mtile
            ksizes = [(0, 128), (128, 128), (256, 16)]
            for mt in range(GPC // MTILE):
                gt = gpool.tile([128, 3, MTILE], F8, tag="gt")
                for t, (k0, kn) in enumerate(ksizes):
                    nc.sync.dma_start(
                        gt[:kn, t, :],
                        gridT[k0:k0 + kn, mt * MTILE:(mt + 1) * MTILE])
                tab = tpool.tile([128, MS, ELEM], BF16, tag="tab")
                nc.vector.memset(tab[:, :, NCH:], 0.0)
                nc.vector.memset(tab[:, :, NCH:NW], WSCALE)
                for mq in range(MS // 4):
                    psum = pt_pool.tile([128, 4, NW], F32, tag="ptab")
                    for q in range(4):
                        ms = mq * 4 + q
                        for t, (k0, kn) in enumerate(ksizes):
                            nc.tensor.matmul(
                                out=psum[:, q, :],
                                lhsT=gt[:kn, t, ms * 128:(ms + 1) * 128],
                                rhs=w_sb[:kn, t * NW:(t + 1) * NW],
                                start=(t == 0), stop=(t == 2))
                    nc.scalar.copy(tab[:, mq * 4:(mq + 1) * 4, :NCH],
                                   psum[:, :, :NCH])
                # p-major table: grid cell c lives at table row
                # (c%128)*64 + c//128, so each partition stores MS
                # consecutive 256B rows in one descriptor
                nc.scalar.dma_start(
                    table[:].rearrange("(p r) e -> p r e", p=128)
                        [:, mt * MS:(mt + 1) * MS, :],
                    tab[:])

            # ---- Phase B: gather + one-hot reduce ----
            for s in range(NSLICE):
                dst = dpool.tile([128, CPS, ELEM], BF16, tag="dst")
                nc.gpsimd.dma_gather(
                    dst[:], table[:], idx_all[:, s, :],
                    IDX_PER_SLICE, IDX_PER_SLICE, ELEM, single_packet=False,
                    queue_num=s % 4)
                psum_w = pb_pool.tile([WIN, WPS, NW], F32, tag="pblk")
                for c in range(CPS):
                    h, j = divmod(c, CPW)
                    nc.tensor.matmul(
                        out=psum_w[:, h, :],
                        lhsT=oh_sb[:, s * CPS + c, :],
                        rhs=dst[:, c, :NW],
                        start=(j == 0), stop=(j == CPW - 1))
                part = papool.tile([WIN, WPS, NW], F16, tag="part")
                nc.scalar.copy(part[:], psum_w[:])
                nc.sync.dma_start(
                    (out_t if skip_collective else partial_d)
                        [:, WPS * s:WPS * (s + 1), :],
                    part[:])

            if not skip_collective:
                nc.gpsimd.collective_compute(
                    "ReduceScatter",
                    mybir.AluOpType.add,
                    replica_groups=[list(range(NCORE))],
                    ins=[partial_d[:]],
                    outs=[rs_out[:]],
                )

            # ---- Phase C: finalize ----
            if not skip_collective:
                fin16 = fpool.tile([8, NWIN, NW], F16, tag="fin16")
                nc.sync.dma_start(fin16[:], rs_out[:])
                fin = fpool.tile([8, NWIN, NW], F32, tag="fin")
                nc.vector.tensor_copy(fin[:], fin16[:])
                cnt = fpool.tile([8, NWIN, 1], F32, tag="cnt")
                nc.vector.tensor_scalar_max(cnt[:], fin[:, :, NCH:NW], 1.0)
                rec = fpool.tile([8, NWIN, 1], F32, tag="rec")
                nc.vector.reciprocal(rec[:], cnt[:])
                sc = fpool.tile([8, NWIN, NCH], F32, tag="sc")
                nc.vector.tensor_tensor(
                    out=sc[:], in0=fin[:, :, :NCH],
                    in1=rec[:].broadcast_to([8, NWIN, NCH]),
                    op=mybir.AluOpType.mult)
                sc2 = fpool.tile([8, NWIN, NCH], F32, tag="sc2")
                nc.vector.tensor_tensor(
                    out=sc2[:], in0=sc[:],
                    in1=bias_sb[:8].rearrange("p (h c) -> p h c", h=1)
                        .broadcast_to([8, NWIN, NCH]),
                    op=mybir.AluOpType.add)
                og = fpool.tile([8, NWIN, NCH], F32, tag="og")
                nc.scalar.activation(og[:], sc2[:],
                                     mybir.ActivationFunctionType.Sigmoid)
                nc.sync.dma_start(out_t[:], og[:])

    nc.compile()
    return nc


def prep_inputs(class_capsules, W, b, point_idx, segment_ids, num_segments=NSEG):
    """Host-side sharding: returns in_maps (list of 8 dicts)."""
    assert int(num_segments) == NSEG
    grid = np.ascontiguousarray(class_capsules.reshape(GRID, D), np.float32)
    point_idx = np.asarray(point_idx, np.int64)
    segment_ids = np.asarray(segment_ids, np.int64)
    W = np.asarray(W, np.float32)
    b = np.asarray(b, np.float32)

    f8 = ml_dtypes.float8_e4m3fn
    w_pack = np.zeros((128, 60), f8)
    w20 = np.concatenate([W, np.zeros((D, 1), np.float32)], 1) * WSCALE
    w_pack[:, 0:20] = w20[0:128].astype(f8)
    w_pack[:, 20:40] = w20[128:256].astype(f8)
    w_pack[0:16, 40:60] = w20[256:272].astype(f8)

    bias_rep = np.tile(b[None, :], (128, 1)).astype(np.float32)

    in_maps = []
    for k in range(NCORE):
        sel = (point_idx >= k * GPC) & (point_idx < (k + 1) * GPC)
        lidx = (point_idx[sel] - k * GPC).astype(np.int64)
        # p-major table layout: cell c lives at row (c%128)*64 + c//128
        lidx = ((lidx % 128) * (GPC // 128) + lidx // 128).astype(np.int16)
        lseg = segment_ids[sel]          # sorted ascending
        win = (lseg >> 6).astype(np.int64)
        srel = (lseg & 63).astype(np.float32)
        counts = np.bincount(win, minlength=NWIN)
        assert counts.max() <= CAP, f"core {k}: window count {counts.max()} > CAP"
        start = np.zeros(NWIN, np.int64)
        start[1:] = np.cumsum(counts)[:-1]
        rank = np.arange(lidx.size) - start[win]
        pos = win * CAP + rank

        idx_pad = np.zeros(NIDX, np.int16)
        srel_pad = np.full(NIDX, -1.0, np.float32)
        idx_pad[pos] = lidx
        srel_pad[pos] = srel

        # wrapped idx, contiguous per partition: [128, NSLICE, 288]
        idxw = np.empty((128, NSLICE, IDX_PER_SLICE // 16), np.int16)
        for s in range(NSLICE):
            chunk = idx_pad[s * IDX_PER_SLICE:(s + 1) * IDX_PER_SLICE]
            idxw[:, s, :] = np.tile(chunk.reshape(-1, 16).T, (8, 1))

        # host-built one-hot: oh[p, g, s] = (srel of slot g*128+p) == s
        sr = srel_pad.reshape(NCHUNK, 128)            # [chunk, slot]
        ohm = (sr.T[:, :, None] ==
               np.arange(WIN, dtype=np.float32)[None, None, :])
        oh = ohm.astype(ml_dtypes.bfloat16)           # [128, NCHUNK, 64]

        gridT_k = np.ascontiguousarray(
            grid[k * GPC:(k + 1) * GPC].T).astype(f8)

        in_maps.append({
            "gridT": gridT_k,
            "w_pack": w_pack,
            "idx": idxw,
            "oh": oh,
            "bias": bias_rep,
        })
    return in_maps


def assemble(results):
    # core k's rs slice holds segments s with s%64 in [8k, 8k+8):
    # out_k[r, w, :] = segment w*64 + 8k + r
    out = np.empty((NSEG, NCH), np.float32)
    for k in range(NCORE):
        res = results[k]["out"]           # [8, 64, 19]
        segs = (np.arange(NWIN)[None, :] * WIN + 8 * k
                + np.arange(8)[:, None])  # [8, 64]
        out[segs.ravel()] = res.reshape(-1, NCH)
    return out


_NC_CACHE = {}


def kernel(class_capsules, W, b, point_idx, segment_ids, num_segments):
    """Full-input entry point: shard across 8 NeuronCores, run, reassemble."""
    from concourse.bass_utils import run_bass_kernel_spmd

    in_maps = prep_inputs(np.asarray(class_capsules), np.asarray(W),
                          np.asarray(b), np.asarray(point_idx),
                          np.asarray(segment_ids), int(num_segments))
    if "nc" not in _NC_CACHE:
        _NC_CACHE["nc"] = build_nc()
    res = run_bass_kernel_spmd(_NC_CACHE["nc"], in_maps, list(range(NCORE)))
    return assemble(res.results)
